# revision 5
# baseline (speedup 1.0000x reference)
"""Multi-head causal attention (B=2, S=2048, D=1024, H=16) on 8 trn2 cores.

Sharding: batch x head-group (2 batches x 4 groups of 4 heads = 8 cores).

Per-core pipeline (all activations bf16, scores matmul fp8 DoubleRow):
  QT/KT = Wq^T x^T, Wk^T x^T   bf16 matmuls -> cast fp8 into a split layout
          qt3/kt3[(h%2)*64+hd, h//2, grp, s] with grp=1 zeroed (DoubleRow
          needs [K, 2, *] operands; the zero band makes group 1 a no-op).
  V     = x @ Wv   bf16, [s, 4 heads, 64+1] with a ones column (denominator).
  S^T   = K Q^T per head via one fp8 DoubleRow matmul per 128-k-tile
          (2x faster than bf16); exp on the Activation engine in pairs of
          k-tiles; causal mask by tri-multiply on GpSimd.
  O     = P^T-as-weights x V in natural [q, hd] orientation (65-wide rhs,
          half the PE cost of the transposed form); softmax normalize via
          strided reciprocal + stride-0 broadcast multiply on DVE.
  O^T   via PE transpose of head-pair blocks [128q, 128dl] (bf16).
  Y     = O^T chunks @ Wo, bf16 matmuls; psum->bf16 copy on GpSimd; DMA out.
Host sums the 4 group partials per batch in f32 and adds bo.

Instruction emission is woven by a greedy two-clock scheduler so the
Activation engine (exp, the near-critical path) is fed early while QKV /
AV / out-proj matmuls fill the PE gaps.
"""

import os

import numpy as np
import ml_dtypes

import concourse.bacc as bacc
import concourse.mybir as mybir
import concourse.tile as tile
from concourse.bass_utils import run_bass_kernel_spmd
from concourse.masks import make_identity, make_upper_triangular

F32 = mybir.dt.float32
BF16 = mybir.dt.bfloat16
FP8 = mybir.dt.float8e4
EXP = mybir.ActivationFunctionType.Exp
DR = mybir.MatmulPerfMode.DoubleRow
MULT = mybir.AluOpType.mult

B, S, D, H, HD = 2, 2048, 1024, 16, 64
P = 128
KD = D // P          # 8 contraction chunks over D
NJ = S // 512        # 4 q-stages of 512
NHL = 4              # heads per core
DL = NHL * HD        # 256 local head dims
N_CORES = 8

PPOOL = int(os.environ.get("K_PPOOL", "18"))
FP8_SCORES = os.environ.get("K_FP8_SCORES", "1") == "1"
LEAD = float(os.environ.get("K_LEAD", "1500"))   # ns of Act run-ahead
XPOOL = int(os.environ.get("K_XPOOL", "3"))


def build_nc(order="G"):
    del order  # single schedule; kept for test.py compat
    nc = bacc.Bacc("TRN2", target_bir_lowering=False, debug=False)
    xT = nc.dram_tensor("xT", [D, S], BF16, kind="ExternalInput")
    wq = nc.dram_tensor("wq", [D, DL], BF16, kind="ExternalInput")
    wk = nc.dram_tensor("wk", [D, DL], BF16, kind="ExternalInput")
    wv = nc.dram_tensor("wv", [D, DL], BF16, kind="ExternalInput")
    wo = nc.dram_tensor("wo", [DL, D], BF16, kind="ExternalInput")
    y = nc.dram_tensor("y", [S, D], BF16, kind="ExternalOutput")

    xT_v = xT.ap().rearrange("(ko p) s -> p ko s", p=P)      # [128, 8, 2048]
    wq_v = wq.ap().rearrange("(ko p) n -> p ko n", p=P)      # [128, 8, 256]
    wk_v = wk.ap().rearrange("(ko p) n -> p ko n", p=P)
    wv_v = wv.ap().rearrange("(ko p) n -> p ko n", p=P)
    wo_v = wo.ap().rearrange("(ko p) n -> p ko n", p=P)      # [128, 2, 1024]
    y_v = y.ap()

    with tile.TileContext(nc) as tc:
        with (
            tc.tile_pool(name="singles", bufs=1) as singles,
            tc.tile_pool(name="xpool", bufs=XPOOL) as xpool,
            tc.tile_pool(name="ppool", bufs=PPOOL) as ppool,
            tc.tile_pool(name="opool", bufs=2) as opool,
            tc.tile_pool(name="ypool", bufs=4) as ypool,
            tc.tile_pool(name="recpool", bufs=8) as recpool,
            tc.tile_pool(name="psum", bufs=1, space="PSUM") as psum,
        ):
            # ---- constants ----
            wq_sb = singles.tile([P, KD, DL], BF16)
            wk_sb = singles.tile([P, KD, DL], BF16)
            wv_sb = singles.tile([P, KD, DL], BF16)
            wo_sb = singles.tile([P, 2, D], BF16)
            nc.sync.dma_start(wq_sb[:], wq_v)
            nc.sync.dma_start(wk_sb[:], wk_v)
            nc.sync.dma_start(wv_sb[:], wv_v)
            nc.gpsimd.dma_start(wo_sb[:], wo_v)

            # qt3/kt3: [part=(h%2)*64+hd, h//2, group, s] fp8, group 1 zeroed
            qkdt = FP8 if FP8_SCORES else BF16
            ngrp = 2 if FP8_SCORES else 1
            qt3 = singles.tile([P, 2, ngrp, S], qkdt)
            kt3 = singles.tile([P, 2, ngrp, S], qkdt)
            if FP8_SCORES:
                nc.gpsimd.memset(qt3[:, :, 1, :], 0.0)
                nc.gpsimd.memset(kt3[:, :, 1, :], 0.0)

            # V' = [V | 1] per (k-tile, head)
            v_sb = singles.tile([P, S // P, NHL, HD + 1], BF16)
            nc.vector.memset(v_sb[:, :, :, HD:HD + 1], 1.0)

            otT_sb = singles.tile([P, 2, S], BF16)   # [dl-in-chunk, chunk, q]

            tri_f = singles.tile([P, P], F32)
            make_upper_triangular(nc, tri_f[:], val=1.0, diag=True)
            tri = singles.tile([P, P], BF16)
            nc.vector.tensor_copy(out=tri[:], in_=tri_f[:])
            ident_f = singles.tile([P, P], F32)
            make_identity(nc, ident_f[:])
            ident = singles.tile([P, P], BF16)
            nc.vector.tensor_copy(out=ident[:], in_=ident_f[:])

            xt_tiles = {}
            plists = {}
            o_tiles = {}

            # ---- step emitters ----
            def emit_x(j):
                sq0 = 512 * j
                xt = xpool.tile([P, KD, 512], BF16, name="xt")
                for k in range(KD):
                    nc.sync.dma_start(xt[:, k, :], xT_v[:, k, sq0:sq0 + 512])
                xt_tiles[j] = xt

            def emit_qk(j, which, m):
                sq0 = 512 * j
                xt = xt_tiles[j]
                w_sb = wq_sb if which == "q" else wk_sb
                dst = qt3 if which == "q" else kt3
                ps = psum.tile([P, 512], F32, tag="w", bufs=2, name="qk_ps")
                for k in range(KD):
                    nc.tensor.matmul(
                        ps[:], w_sb[:, k, 128 * m:128 * m + 128], xt[:, k, :],
                        start=(k == 0), stop=(k == KD - 1))
                nc.vector.tensor_copy(out=dst[:, m, 0, sq0:sq0 + 512],
                                      in_=ps[:])

            def emit_v(j, t):
                xt = xt_tiles[j]
                ps = psum.tile([P, DL], F32, tag="w", bufs=2, name="v_ps")
                for k in range(KD):
                    nc.tensor.matmul(
                        ps[:], xt[:, k, 128 * t:128 * t + 128], wv_sb[:, k, :],
                        start=(k == 0), stop=(k == KD - 1))
                nc.vector.tensor_copy(
                    out=v_sb[:, 4 * j + t, :, 0:HD],
                    in_=ps.rearrange("p (h d) -> p h d", h=NHL))

            def emit_s(j, pr, ip):
                sq0 = 512 * j
                sq = psum.tile([P, 2, 2, 512], F32, tag="s", bufs=1, name="sq")
                p2 = ppool.tile([P, 2, 2, 512], BF16, name="p2")
                c0s = []
                for isel, i in enumerate((2 * ip, 2 * ip + 1)):
                    r0 = 128 * i - sq0
                    c0 = min(max(r0, 0), 384)
                    c0s.append((isel, i, r0, c0))
                    for hh in range(2):
                        h = 2 * pr + hh
                        base = 64 * (h % 2)
                        hp = h // 2
                        if FP8_SCORES:
                            nc.tensor.matmul(
                                sq[:, isel, hh, c0:512],
                                kt3[base:base + 64, hp, :,
                                    128 * i:128 * i + 128],
                                qt3[base:base + 64, hp, :,
                                    sq0 + c0:sq0 + 512],
                                start=True, stop=True, perf_mode=DR)
                        else:
                            nc.tensor.matmul(
                                sq[:, isel, hh, c0:512],
                                kt3[base:base + 64, hp, 0,
                                    128 * i:128 * i + 128],
                                qt3[base:base + 64, hp, 0,
                                    sq0 + c0:sq0 + 512],
                                start=True, stop=True)
                if c0s[0][3] == c0s[1][3]:
                    c0lo = c0s[0][3]
                    nc.scalar.activation(p2[:, :, :, c0lo:512],
                                         sq[:, :, :, c0lo:512], EXP,
                                         scale=0.125)
                else:
                    for isel, i, r0, c0 in c0s:
                        nc.scalar.activation(p2[:, isel, :, c0:512],
                                             sq[:, isel, :, c0:512], EXP,
                                             scale=0.125)
                for isel, i, r0, c0 in c0s:
                    if r0 >= 0:
                        for hh in range(2):
                            nc.gpsimd.tensor_mul(
                                out=p2[:, isel, hh, r0:r0 + 128],
                                in0=p2[:, isel, hh, r0:r0 + 128], in1=tri[:])
                plists[(j, pr)].append(p2)

            def emit_a(j, pr, t):
                sq0 = 512 * j
                T = 4 * j + t
                plist = plists[(j, pr)]
                if t == 0:
                    o_tiles[(j, pr)] = opool.tile([P, 4, 2, HD], BF16,
                                                  name="o_sb")
                o_sb = o_tiles[(j, pr)]
                u2 = psum.tile([P, 2, HD + 1], F32, tag="u", bufs=2, name="u2")
                for hh in range(2):
                    h = 2 * pr + hh
                    for i in range(T + 1):
                        nc.tensor.matmul(
                            u2[:, hh, :],
                            plist[i // 2][:, i % 2, hh, 128 * t:128 * t + 128],
                            v_sb[:, i, h, :],
                            start=(i == 0), stop=(i == T))
                rec = recpool.tile([P, 2], F32, name="rec")
                nc.vector.reciprocal(out=rec[:], in_=u2[:, :, HD])
                nc.vector.tensor_tensor(
                    out=o_sb[:, t, :, :], in0=u2[:, :, 0:HD],
                    in1=rec[:, :, None].broadcast_to([P, 2, HD]), op=MULT)
                tp = psum.tile([P, P], BF16, tag="w", bufs=2, name="tp")
                nc.tensor.transpose(tp[:], o_sb[:, t, :, :], ident[:])
                nc.vector.tensor_copy(
                    out=otT_sb[:, pr, sq0 + 128 * t:sq0 + 128 * t + 128],
                    in_=tp[:])

            def emit_o(j, t):
                sq0 = 512 * j
                q0 = sq0 + 128 * t
                for n in range(2):
                    yps = psum.tile([P, 512], F32, tag="w", bufs=2,
                                    name="y_ps")
                    for k in range(2):
                        nc.tensor.matmul(
                            yps[:], otT_sb[:, k, q0:q0 + 128],
                            wo_sb[:, k, 512 * n:512 * n + 512],
                            start=(k == 0), stop=(k == 1))
                    ysb = ypool.tile([P, 512], BF16, name="y_sb")
                    nc.vector.tensor_copy(out=ysb[:], in_=yps[:])
                    nc.sync.dma_start(y_v[q0:q0 + 128, 512 * n:512 * n + 512],
                                      ysb[:])

            # ---- cost model for the weave (ns) ----
            PE_C = 0.4167
            ACT_C = 0.8333

            def s_cost(j, ip):
                pe = act = 0.0
                for i in (2 * ip, 2 * ip + 1):
                    c0 = min(max(128 * i - 512 * j, 0), 384)
                    pe += (512 - c0) * PE_C * (1.0 if FP8_SCORES else 2.0)
                    act += 2 * (512 - c0) * ACT_C
                return pe, act + 185.0

            def a_cost(j, t):
                return (2 * (4 * j + t + 1) * 65 + 128) * PE_C

            # ---- build step list ----
            # filler queue: x/qkv in fixed order; a/o appended when unlocked
            fq = []
            for j in range(NJ):
                fq.append(("x", j))
                for m in range(2):
                    fq.append(("qk", j, "q", m))
                    fq.append(("qk", j, "k", m))
                for t in range(4):
                    fq.append(("v", j, t))
            s_units = [(j, pr) for j in range(NJ) for pr in range(2)]
            for u in s_units:
                plists[u] = []

            def emit(step):
                kind = step[0]
                if kind == "x":
                    emit_x(step[1])
                elif kind == "qk":
                    emit_qk(step[1], step[2], step[3])
                elif kind == "v":
                    emit_v(step[1], step[2])
                elif kind == "s":
                    emit_s(step[1], step[2], step[3])
                elif kind == "a":
                    emit_a(step[1], step[2], step[3])
                elif kind == "o":
                    emit_o(step[1], step[2])

            pe_t = 0.0
            act_t = 0.0
            fq_i = 0
            qkv_emitted = [0] * NJ        # chains emitted per j (10 = done)
            s_idx = 0                     # index into flat s-step list
            s_steps = []
            for (j, pr) in s_units:
                for ip in range(2 * j + 2):
                    s_steps.append((j, pr, ip))
            s_done_per_unit = {u: 0 for u in s_units}
            a_emitted = {}
            o_appended = set()

            def s_eligible(idx):
                if idx >= len(s_steps):
                    return False
                j, pr, ip = s_steps[idx]
                # needs qt/kt of stages 0..j fully projected
                if any(qkv_emitted[jj] < 10 for jj in range(j + 1)):
                    return False
                # don't run >2 units ahead of AV consumption (ppool safety)
                ui = s_units.index((j, pr))
                if ui >= 2 and a_emitted.get(s_units[ui - 2], 0) < 4:
                    return False
                return True

            def append_unlocked_fillers():
                # a-steps unlock as their s prefix lands; o after both prs
                for (j, pr) in s_units:
                    done_ip = s_done_per_unit[(j, pr)]
                    a_n = a_emitted.get((j, pr), 0)
                    while a_n < 4 and done_ip * 2 >= (4 * j + a_n) + 1:
                        fq.append(("a", j, pr, a_n))
                        a_n += 1
                        a_emitted[(j, pr)] = a_n
                for j in range(NJ):
                    for t in range(4):
                        if (j, t) in o_appended:
                            continue
                        if (a_emitted.get((j, 0), 0) > t
                                and a_emitted.get((j, 1), 0) > t):
                            fq.append(("o", j, t))
                            o_appended.add((j, t))

            while s_idx < len(s_steps) or fq_i < len(fq):
                append_unlocked_fillers()
                el = s_eligible(s_idx)
                if el and act_t <= pe_t + LEAD:
                    j, pr, ip = s_steps[s_idx]
                    pe, act = s_cost(j, ip)
                    emit(("s", j, pr, ip))
                    act_t = max(act_t, pe_t + pe) + act
                    pe_t += pe
                    s_done_per_unit[(j, pr)] += 1
                    s_idx += 1
                elif fq_i < len(fq):
                    step = fq[fq_i]
                    fq_i += 1
                    emit(step)
                    if step[0] == "qk":
                        pe_t += 8 * 512 * PE_C
                        qkv_emitted[step[1]] += 3
                    elif step[0] == "v":
                        pe_t += 8 * 256 * PE_C
                        qkv_emitted[step[1]] += 1
                        if step[2] == 3:
                            qkv_emitted[step[1]] = max(
                                qkv_emitted[step[1]], 10)
                    elif step[0] == "a":
                        pe_t += a_cost(step[1], step[3])
                    elif step[0] == "o":
                        pe_t += 2048 * PE_C
                elif el:
                    j, pr, ip = s_steps[s_idx]
                    pe, act = s_cost(j, ip)
                    emit(("s", j, pr, ip))
                    act_t = max(act_t, pe_t + pe) + act
                    pe_t += pe
                    s_done_per_unit[(j, pr)] += 1
                    s_idx += 1
                else:
                    raise RuntimeError("scheduler stuck")
            append_unlocked_fillers()
            while fq_i < len(fq):
                emit(fq[fq_i])
                fq_i += 1
                append_unlocked_fillers()
    nc.finalize()
    return nc


_NC_CACHE = []


def _shard_inputs(inputs):
    bf = ml_dtypes.bfloat16
    x = np.asarray(inputs["x"], dtype=np.float32)
    Wq = np.asarray(inputs["Wq"], dtype=np.float32)
    Wk = np.asarray(inputs["Wk"], dtype=np.float32)
    Wv = np.asarray(inputs["Wv"], dtype=np.float32)
    Wo = np.asarray(inputs["Wo"], dtype=np.float32)
    in_maps = []
    for c in range(N_CORES):
        b, g = divmod(c, N_CORES // B)
        cols = slice(g * DL, (g + 1) * DL)
        in_maps.append({
            "xT": np.ascontiguousarray(x[b].T).astype(bf),
            "wq": np.ascontiguousarray(Wq[:, cols]).astype(bf),
            "wk": np.ascontiguousarray(Wk[:, cols]).astype(bf),
            "wv": np.ascontiguousarray(Wv[:, cols]).astype(bf),
            "wo": np.ascontiguousarray(Wo[cols, :]).astype(bf),
        })
    return in_maps


def kernel(**inputs) -> np.ndarray:
    bo = np.asarray(inputs["bo"], dtype=np.float32)
    in_maps = _shard_inputs(inputs)

    if not _NC_CACHE:
        _NC_CACHE.append(build_nc())
    nc = _NC_CACHE[0]
    res = run_bass_kernel_spmd(nc, in_maps, core_ids=list(range(N_CORES)))
    ys = [np.asarray(r["y"], dtype=np.float32) for r in res.results]
    gpb = N_CORES // B
    out = np.stack([
        np.sum(ys[b * gpb:(b + 1) * gpb], axis=0) + bo for b in range(B)
    ]).astype(np.float32)
    return out


# revision 6
# speedup vs baseline: 1.0901x; 1.0901x over previous
"""Multi-head causal attention (B=2, S=2048, D=1024, H=16) on 8 trn2 cores.

Sharding: batch x head-group (2 batches x 4 groups of 4 heads = 8 cores).

Per-core pipeline (all activations bf16, scores matmul fp8 DoubleRow):
  QT/KT = Wq^T x^T, Wk^T x^T   bf16 matmuls -> cast fp8 into a split layout
          qt3/kt3[(h%2)*64+hd, h//2, grp, s] with grp=1 zeroed (DoubleRow
          needs [K, 2, *] operands; the zero band makes group 1 a no-op).
  V     = x @ Wv   bf16, [s, 4 heads, 64+1] with a ones column (denominator).
  S^T   = K Q^T per head via one fp8 DoubleRow matmul per 128-k-tile
          (2x faster than bf16); exp on the Activation engine in pairs of
          k-tiles; causal mask by tri-multiply on GpSimd.
  O     = P^T-as-weights x V in natural [q, hd] orientation (65-wide rhs,
          half the PE cost of the transposed form); softmax normalize via
          strided reciprocal + stride-0 broadcast multiply on DVE.
  O^T   via PE transpose of head-pair blocks [128q, 128dl] (bf16).
  Y     = O^T chunks @ Wo, bf16 matmuls; psum->bf16 copy on GpSimd; DMA out.
Host sums the 4 group partials per batch in f32 and adds bo.

Instruction emission is woven by a greedy two-clock scheduler so the
Activation engine (exp, the near-critical path) is fed early while QKV /
AV / out-proj matmuls fill the PE gaps.
"""

import os

import numpy as np
import ml_dtypes

import concourse.bacc as bacc
import concourse.mybir as mybir
import concourse.tile as tile
from concourse.bass_utils import run_bass_kernel_spmd
from concourse.masks import make_identity, make_upper_triangular

F32 = mybir.dt.float32
BF16 = mybir.dt.bfloat16
FP8 = mybir.dt.float8e4
EXP = mybir.ActivationFunctionType.Exp
DR = mybir.MatmulPerfMode.DoubleRow
MULT = mybir.AluOpType.mult

B, S, D, H, HD = 2, 2048, 1024, 16, 64
P = 128
KD = D // P          # 8 contraction chunks over D
NJ = S // 512        # 4 q-stages of 512
NHL = 4              # heads per core
DL = NHL * HD        # 256 local head dims
N_CORES = 8

PPOOL = int(os.environ.get("K_PPOOL", "18"))
FP8_SCORES = os.environ.get("K_FP8_SCORES", "1") == "1"
LEAD = float(os.environ.get("K_LEAD", "1500"))   # ns of Act run-ahead
XPOOL = int(os.environ.get("K_XPOOL", "3"))
SEQ = os.environ.get("K_SEQ", "0") == "1"


def build_nc(order="G"):
    del order  # single schedule; kept for test.py compat
    nc = bacc.Bacc("TRN2", target_bir_lowering=False, debug=False)
    xT = nc.dram_tensor("xT", [D, S], BF16, kind="ExternalInput")
    wq = nc.dram_tensor("wq", [D, DL], BF16, kind="ExternalInput")
    wk = nc.dram_tensor("wk", [D, DL], BF16, kind="ExternalInput")
    wv = nc.dram_tensor("wv", [D, DL], BF16, kind="ExternalInput")
    wo = nc.dram_tensor("wo", [DL, D], BF16, kind="ExternalInput")
    y = nc.dram_tensor("y", [S, D], BF16, kind="ExternalOutput")

    xT_v = xT.ap().rearrange("(ko p) s -> p ko s", p=P)      # [128, 8, 2048]
    wq_v = wq.ap().rearrange("(ko p) n -> p ko n", p=P)      # [128, 8, 256]
    wk_v = wk.ap().rearrange("(ko p) n -> p ko n", p=P)
    wv_v = wv.ap().rearrange("(ko p) n -> p ko n", p=P)
    wo_v = wo.ap().rearrange("(ko p) n -> p ko n", p=P)      # [128, 2, 1024]
    y_v = y.ap()

    with tile.TileContext(nc) as tc:
        with (
            tc.tile_pool(name="singles", bufs=1) as singles,
            tc.tile_pool(name="xpool", bufs=XPOOL) as xpool,
            tc.tile_pool(name="ppool", bufs=PPOOL) as ppool,
            tc.tile_pool(name="opool", bufs=2) as opool,
            tc.tile_pool(name="ypool", bufs=4) as ypool,
            tc.tile_pool(name="recpool", bufs=8) as recpool,
            tc.tile_pool(name="psum", bufs=1, space="PSUM") as psum,
        ):
            # ---- constants ----
            wq_sb = singles.tile([P, KD, DL], BF16)
            wk_sb = singles.tile([P, KD, DL], BF16)
            wv_sb = singles.tile([P, KD, DL], BF16)
            wo_sb = singles.tile([P, 2, D], BF16)
            nc.sync.dma_start(wq_sb[:], wq_v)
            nc.sync.dma_start(wk_sb[:], wk_v)
            nc.sync.dma_start(wv_sb[:], wv_v)
            nc.gpsimd.dma_start(wo_sb[:], wo_v)

            # qt3/kt3: [part=(h%2)*64+hd, h//2, group, s] fp8, group 1 zeroed
            qkdt = FP8 if FP8_SCORES else BF16
            ngrp = 2 if FP8_SCORES else 1
            qt3 = singles.tile([P, 2, ngrp, S], qkdt)
            kt3 = singles.tile([P, 2, ngrp, S], qkdt)
            if FP8_SCORES:
                nc.gpsimd.memset(qt3[:, :, 1, :], 0.0)
                nc.gpsimd.memset(kt3[:, :, 1, :], 0.0)

            # V' = [V | 1] per (k-tile, head)
            v_sb = singles.tile([P, S // P, NHL, HD + 1], BF16)
            nc.vector.memset(v_sb[:, :, :, HD:HD + 1], 1.0)

            otT_sb = singles.tile([P, 2, S], BF16)   # [dl-in-chunk, chunk, q]

            tri_f = singles.tile([P, P], F32)
            make_upper_triangular(nc, tri_f[:], val=1.0, diag=True)
            tri = singles.tile([P, P], BF16)
            nc.vector.tensor_copy(out=tri[:], in_=tri_f[:])
            ident_f = singles.tile([P, P], F32)
            make_identity(nc, ident_f[:])
            ident = singles.tile([P, P], BF16)
            nc.vector.tensor_copy(out=ident[:], in_=ident_f[:])

            xt_tiles = {}
            plists = {}
            o_tiles = {}

            # ---- step emitters ----
            def emit_x(j):
                sq0 = 512 * j
                xt = xpool.tile([P, KD, 512], BF16, name="xt")
                for k in range(KD):
                    nc.sync.dma_start(xt[:, k, :], xT_v[:, k, sq0:sq0 + 512])
                xt_tiles[j] = xt

            def emit_qk(j, which, m):
                sq0 = 512 * j
                xt = xt_tiles[j]
                w_sb = wq_sb if which == "q" else wk_sb
                dst = qt3 if which == "q" else kt3
                ps = psum.tile([P, 512], F32, tag="w", bufs=2, name="qk_ps")
                for k in range(KD):
                    nc.tensor.matmul(
                        ps[:], w_sb[:, k, 128 * m:128 * m + 128], xt[:, k, :],
                        start=(k == 0), stop=(k == KD - 1))
                nc.vector.tensor_copy(out=dst[:, m, 0, sq0:sq0 + 512],
                                      in_=ps[:])

            def emit_v(j, t):
                xt = xt_tiles[j]
                ps = psum.tile([P, DL], F32, tag="w", bufs=2, name="v_ps")
                for k in range(KD):
                    nc.tensor.matmul(
                        ps[:], xt[:, k, 128 * t:128 * t + 128], wv_sb[:, k, :],
                        start=(k == 0), stop=(k == KD - 1))
                nc.vector.tensor_copy(
                    out=v_sb[:, 4 * j + t, :, 0:HD],
                    in_=ps.rearrange("p (h d) -> p h d", h=NHL))

            def emit_s(j, pr, ip):
                sq0 = 512 * j
                sq = psum.tile([P, 2, 2, 512], F32, tag="s", bufs=1, name="sq")
                p2 = ppool.tile([P, 2, 2, 512], BF16, name="p2")
                c0s = []
                for isel, i in enumerate((2 * ip, 2 * ip + 1)):
                    r0 = 128 * i - sq0
                    c0 = min(max(r0, 0), 384)
                    c0s.append((isel, i, r0, c0))
                    for hh in range(2):
                        h = 2 * pr + hh
                        base = 64 * (h % 2)
                        hp = h // 2
                        if FP8_SCORES:
                            nc.tensor.matmul(
                                sq[:, isel, hh, c0:512],
                                kt3[base:base + 64, hp, :,
                                    128 * i:128 * i + 128],
                                qt3[base:base + 64, hp, :,
                                    sq0 + c0:sq0 + 512],
                                start=True, stop=True, perf_mode=DR)
                        else:
                            nc.tensor.matmul(
                                sq[:, isel, hh, c0:512],
                                kt3[base:base + 64, hp, 0,
                                    128 * i:128 * i + 128],
                                qt3[base:base + 64, hp, 0,
                                    sq0 + c0:sq0 + 512],
                                start=True, stop=True)
                if c0s[0][3] == c0s[1][3]:
                    c0lo = c0s[0][3]
                    nc.scalar.activation(p2[:, :, :, c0lo:512],
                                         sq[:, :, :, c0lo:512], EXP,
                                         scale=0.125)
                else:
                    for isel, i, r0, c0 in c0s:
                        nc.scalar.activation(p2[:, isel, :, c0:512],
                                             sq[:, isel, :, c0:512], EXP,
                                             scale=0.125)
                for isel, i, r0, c0 in c0s:
                    if r0 >= 0:
                        for hh in range(2):
                            nc.gpsimd.tensor_mul(
                                out=p2[:, isel, hh, r0:r0 + 128],
                                in0=p2[:, isel, hh, r0:r0 + 128], in1=tri[:])
                plists[(j, pr)].append(p2)

            def emit_a(j, pr, t):
                sq0 = 512 * j
                T = 4 * j + t
                plist = plists[(j, pr)]
                if t == 0:
                    o_tiles[(j, pr)] = opool.tile([P, 4, 2, HD], BF16,
                                                  name="o_sb")
                o_sb = o_tiles[(j, pr)]
                u2 = psum.tile([P, 2, HD + 1], F32, tag="u", bufs=2, name="u2")
                for hh in range(2):
                    h = 2 * pr + hh
                    for i in range(T + 1):
                        nc.tensor.matmul(
                            u2[:, hh, :],
                            plist[i // 2][:, i % 2, hh, 128 * t:128 * t + 128],
                            v_sb[:, i, h, :],
                            start=(i == 0), stop=(i == T))
                rec = recpool.tile([P, 2], F32, name="rec")
                nc.vector.reciprocal(out=rec[:], in_=u2[:, :, HD])
                nc.vector.tensor_tensor(
                    out=o_sb[:, t, :, :], in0=u2[:, :, 0:HD],
                    in1=rec[:, :, None].broadcast_to([P, 2, HD]), op=MULT)
                tp = psum.tile([P, P], BF16, tag="w", bufs=2, name="tp")
                nc.tensor.transpose(tp[:], o_sb[:, t, :, :], ident[:])
                nc.vector.tensor_copy(
                    out=otT_sb[:, pr, sq0 + 128 * t:sq0 + 128 * t + 128],
                    in_=tp[:])

            def emit_o(j, t):
                sq0 = 512 * j
                q0 = sq0 + 128 * t
                for n in range(2):
                    yps = psum.tile([P, 512], F32, tag="w", bufs=2,
                                    name="y_ps")
                    for k in range(2):
                        nc.tensor.matmul(
                            yps[:], otT_sb[:, k, q0:q0 + 128],
                            wo_sb[:, k, 512 * n:512 * n + 512],
                            start=(k == 0), stop=(k == 1))
                    ysb = ypool.tile([P, 512], BF16, name="y_sb")
                    nc.vector.tensor_copy(out=ysb[:], in_=yps[:])
                    nc.sync.dma_start(y_v[q0:q0 + 128, 512 * n:512 * n + 512],
                                      ysb[:])

            # ---- cost model for the weave (ns) ----
            PE_C = 0.4167
            ACT_C = 0.8333

            def s_cost(j, ip):
                pe = act = 0.0
                for i in (2 * ip, 2 * ip + 1):
                    c0 = min(max(128 * i - 512 * j, 0), 384)
                    pe += (512 - c0) * PE_C * (1.0 if FP8_SCORES else 2.0)
                    act += 2 * (512 - c0) * ACT_C
                return pe, act + 185.0

            def a_cost(j, t):
                return (2 * (4 * j + t + 1) * 65 + 128) * PE_C

            # ---- build step list ----
            # filler queue: x/qkv in fixed order; a/o appended when unlocked
            fq = []
            for j in range(NJ):
                fq.append(("x", j))
                for m in range(2):
                    fq.append(("qk", j, "q", m))
                    fq.append(("qk", j, "k", m))
                for t in range(4):
                    fq.append(("v", j, t))
            s_units = [(j, pr) for j in range(NJ) for pr in range(2)]
            for u in s_units:
                plists[u] = []

            def emit(step):
                kind = step[0]
                if kind == "x":
                    emit_x(step[1])
                elif kind == "qk":
                    emit_qk(step[1], step[2], step[3])
                elif kind == "v":
                    emit_v(step[1], step[2])
                elif kind == "s":
                    emit_s(step[1], step[2], step[3])
                elif kind == "a":
                    emit_a(step[1], step[2], step[3])
                elif kind == "o":
                    emit_o(step[1], step[2])

            pe_t = 0.0
            act_t = 0.0
            fq_i = 0
            qkv_emitted = [0] * NJ        # chains emitted per j (10 = done)
            s_idx = 0                     # index into flat s-step list
            s_steps = []
            for (j, pr) in s_units:
                for ip in range(2 * j + 2):
                    s_steps.append((j, pr, ip))
            s_done_per_unit = {u: 0 for u in s_units}
            a_emitted = {}
            o_appended = set()

            def s_eligible(idx):
                if idx >= len(s_steps):
                    return False
                j, pr, ip = s_steps[idx]
                # needs qt/kt of stages 0..j fully projected
                if any(qkv_emitted[jj] < 10 for jj in range(j + 1)):
                    return False
                # don't run >2 units ahead of AV consumption (ppool safety)
                ui = s_units.index((j, pr))
                if ui >= 2 and a_emitted.get(s_units[ui - 2], 0) < 4:
                    return False
                return True

            def append_unlocked_fillers():
                # a-steps unlock as their s prefix lands; o after both prs
                for (j, pr) in s_units:
                    done_ip = s_done_per_unit[(j, pr)]
                    a_n = a_emitted.get((j, pr), 0)
                    while a_n < 4 and done_ip * 2 >= (4 * j + a_n) + 1:
                        fq.append(("a", j, pr, a_n))
                        a_n += 1
                        a_emitted[(j, pr)] = a_n
                for j in range(NJ):
                    for t in range(4):
                        if (j, t) in o_appended:
                            continue
                        if (a_emitted.get((j, 0), 0) > t
                                and a_emitted.get((j, 1), 0) > t):
                            fq.append(("o", j, t))
                            o_appended.add((j, t))

            if SEQ:
                for j in range(NJ):
                    emit(("x", j))
                    for m in range(2):
                        emit(("qk", j, "q", m))
                        emit(("qk", j, "k", m))
                    for t in range(4):
                        emit(("v", j, t))
                for j in range(NJ):
                    for pr in range(2):
                        for ip in range(2 * j + 2):
                            emit(("s", j, pr, ip))
                        for t in range(4):
                            emit(("a", j, pr, t))
                    for t in range(4):
                        emit(("o", j, t))
                s_idx = len(s_steps)
                fq_i = len(fq) + 10**9
            while s_idx < len(s_steps) or fq_i < len(fq):
                append_unlocked_fillers()
                el = s_eligible(s_idx)
                if el and act_t <= pe_t + LEAD:
                    j, pr, ip = s_steps[s_idx]
                    pe, act = s_cost(j, ip)
                    emit(("s", j, pr, ip))
                    act_t = max(act_t, pe_t + pe) + act
                    pe_t += pe
                    s_done_per_unit[(j, pr)] += 1
                    s_idx += 1
                elif fq_i < len(fq):
                    step = fq[fq_i]
                    fq_i += 1
                    emit(step)
                    if step[0] == "qk":
                        pe_t += 8 * 512 * PE_C
                        qkv_emitted[step[1]] += 3
                    elif step[0] == "v":
                        pe_t += 8 * 256 * PE_C
                        qkv_emitted[step[1]] += 1
                        if step[2] == 3:
                            qkv_emitted[step[1]] = max(
                                qkv_emitted[step[1]], 10)
                    elif step[0] == "a":
                        pe_t += a_cost(step[1], step[3])
                    elif step[0] == "o":
                        pe_t += 2048 * PE_C
                elif el:
                    j, pr, ip = s_steps[s_idx]
                    pe, act = s_cost(j, ip)
                    emit(("s", j, pr, ip))
                    act_t = max(act_t, pe_t + pe) + act
                    pe_t += pe
                    s_done_per_unit[(j, pr)] += 1
                    s_idx += 1
                else:
                    raise RuntimeError("scheduler stuck")
            if not SEQ:
                append_unlocked_fillers()
                while fq_i < len(fq):
                    emit(fq[fq_i])
                    fq_i += 1
                    append_unlocked_fillers()
    nc.finalize()
    return nc


_NC_CACHE = []


def _shard_inputs(inputs):
    bf = ml_dtypes.bfloat16
    x = np.asarray(inputs["x"], dtype=np.float32)
    Wq = np.asarray(inputs["Wq"], dtype=np.float32)
    Wk = np.asarray(inputs["Wk"], dtype=np.float32)
    Wv = np.asarray(inputs["Wv"], dtype=np.float32)
    Wo = np.asarray(inputs["Wo"], dtype=np.float32)
    in_maps = []
    for c in range(N_CORES):
        b, g = divmod(c, N_CORES // B)
        cols = slice(g * DL, (g + 1) * DL)
        in_maps.append({
            "xT": np.ascontiguousarray(x[b].T).astype(bf),
            "wq": np.ascontiguousarray(Wq[:, cols]).astype(bf),
            "wk": np.ascontiguousarray(Wk[:, cols]).astype(bf),
            "wv": np.ascontiguousarray(Wv[:, cols]).astype(bf),
            "wo": np.ascontiguousarray(Wo[cols, :]).astype(bf),
        })
    return in_maps


def kernel(**inputs) -> np.ndarray:
    bo = np.asarray(inputs["bo"], dtype=np.float32)
    in_maps = _shard_inputs(inputs)

    if not _NC_CACHE:
        _NC_CACHE.append(build_nc())
    nc = _NC_CACHE[0]
    res = run_bass_kernel_spmd(nc, in_maps, core_ids=list(range(N_CORES)))
    ys = [np.asarray(r["y"], dtype=np.float32) for r in res.results]
    gpb = N_CORES // B
    out = np.stack([
        np.sum(ys[b * gpb:(b + 1) * gpb], axis=0) + bo for b in range(B)
    ]).astype(np.float32)
    return out
